# revision 3
# baseline (speedup 1.0000x reference)
"""LlamaMlpWithLora on 8 Trainium2 NeuronCores.

Strategy: LoRA adapters are merged into the base weights on the host
(W_eff[a] = W + wa[a] @ wb[a], exact math), so the device runs a pure
segmented dense MLP: gate/up GEMM + silu*mult + down GEMM with per-segment
weights. seg_ids are sorted, so tokens form A contiguous segments; each
segment is processed in ragged chunks of <=512 tokens (the chunk pattern is
global, so all 8 cores run an identical SPMD program).

Sharding: tensor-parallel over the intermediate dim (each core owns 1376
real I-columns, padded to 1408 = 11 tiles of 128). The down projection uses
tokens as the moving dim (out = [128 H-partitions, n tokens]) so ragged
chunks are plain column ranges. Each core writes a full [H, T] bf16 down
partial; the host sums the 8 partials and transposes.
"""

import sys

sys.path.insert(0, "/opt/trn_rl_repo")

import numpy as np
import ml_dtypes

T, H, I, R, A = 4096, 4096, 11008, 16, 4
NC_CORES = 8
ISR = I // NC_CORES      # 1376 real intermediate columns per core
NIT = -(-ISR // 128)     # 11 i-tiles per core (last one zero-padded)
ISP = NIT * 128          # 1408
NKT = H // 128           # 32 contraction tiles over hidden dim
NHT = H // 128           # 32 output tiles of the down projection
CHUNK = 512              # max tokens per PSUM accumulation group

_cached = {}             # segs-key -> compiled program
_last_key = None


def _segments(seg_ids):
    """[(adapter, start, [chunk sizes]), ...] — compile-time constants."""
    seg_ids = np.asarray(seg_ids)
    bounds = [0] + [int(b) for b in np.flatnonzero(np.diff(seg_ids)) + 1] + [T]
    segs = []
    for b0, b1 in zip(bounds[:-1], bounds[1:]):
        L = b1 - b0
        n = -(-L // CHUNK)
        base, rem = divmod(L, n)
        sizes = tuple(int(base + (1 if j < rem else 0)) for j in range(n))
        segs.append((int(seg_ids[b0]), int(b0), sizes))
    return tuple(segs)


def _build_program(segs):
    import concourse.bass as bass
    import concourse.tile as tile
    from concourse import bacc, mybir

    bf = mybir.dt.bfloat16
    f32 = mybir.dt.float32
    mult = mybir.AluOpType.mult
    ds = bass.ds
    silu = mybir.ActivationFunctionType.Silu

    lmax = max(sum(sizes) for _, _, sizes in segs)

    nc = bacc.Bacc("TRN2", target_bir_lowering=False, debug=False,
                   num_devices=NC_CORES)

    # DRAM inputs, tiled on host so every DMA slice is contiguous-ish.
    xt = nc.dram_tensor("xt", [128, NKT, T], bf, kind="ExternalInput")
    gwt = nc.dram_tensor("gwt", [A * NIT * 128, NKT, 128], bf,
                         kind="ExternalInput")
    uwt = nc.dram_tensor("uwt", [A * NIT * 128, NKT, 128], bf,
                         kind="ExternalInput")
    dwt = nc.dram_tensor("dwt", [A * NHT * 128, NIT, 128], bf,
                         kind="ExternalInput")
    out = nc.dram_tensor("out", [H, T], bf, kind="ExternalOutput")

    with tile.TileContext(nc) as tc:
        with (
            tc.tile_pool(name="x", bufs=1) as xpool,
            tc.tile_pool(name="act", bufs=2) as actpool,
            tc.tile_pool(name="w", bufs=4) as wpool,
            tc.tile_pool(name="dw", bufs=2) as dwpool,
            tc.tile_pool(name="tmp", bufs=2) as tmppool,
            tc.tile_pool(name="o", bufs=4) as opool,
            tc.tile_pool(name="psg", bufs=2, space="PSUM") as psg,
            tc.tile_pool(name="psu", bufs=2, space="PSUM") as psu,
            tc.tile_pool(name="psd", bufs=2, space="PSUM") as psd,
        ):
            for a, s0, sizes in segs:
                L = sum(sizes)
                x_sb = xpool.tile([128, NKT, lmax], bf, tag="x")
                nc.sync.dma_start(x_sb[:, :, ds(0, L)], xt[:, :, ds(s0, L)])
                act_sb = actpool.tile([128, NIT, lmax], bf, tag="act")

                for io in range(NIT):
                    gw_sb = wpool.tile([128, NKT, 128], bf, tag="w")
                    nc.sync.dma_start(
                        gw_sb[:], gwt[ds((a * NIT + io) * 128, 128), :, :])
                    uw_sb = wpool.tile([128, NKT, 128], bf, tag="w")
                    nc.sync.dma_start(
                        uw_sb[:], uwt[ds((a * NIT + io) * 128, 128), :, :])
                    c0 = 0
                    for n in sizes:
                        pg = psg.tile([128, CHUNK], f32, tag="pg")
                        for k in range(NKT):
                            nc.tensor.matmul(pg[:, ds(0, n)],
                                             gw_sb[:, k, :],
                                             x_sb[:, k, ds(c0, n)],
                                             start=(k == 0),
                                             stop=(k == NKT - 1))
                        pu = psu.tile([128, CHUNK], f32, tag="pu")
                        for k in range(NKT):
                            nc.tensor.matmul(pu[:, ds(0, n)],
                                             uw_sb[:, k, :],
                                             x_sb[:, k, ds(c0, n)],
                                             start=(k == 0),
                                             stop=(k == NKT - 1))
                        tmp = tmppool.tile([128, CHUNK], f32, tag="tmp")
                        nc.scalar.activation(tmp[:, ds(0, n)], pg[:, ds(0, n)],
                                             silu)
                        nc.vector.tensor_tensor(act_sb[:, io, ds(c0, n)],
                                                tmp[:, ds(0, n)],
                                                pu[:, ds(0, n)], mult)
                        c0 += n

                for h in range(NHT):
                    dw_sb = dwpool.tile([128, NIT, 128], bf, tag="dw")
                    nc.scalar.dma_start(
                        dw_sb[:], dwt[ds((a * NHT + h) * 128, 128), :, :])
                    c0 = 0
                    for n in sizes:
                        pd = psd.tile([128, CHUNK], f32, tag="pd")
                        for io in range(NIT):
                            nc.tensor.matmul(pd[:, ds(0, n)],
                                             dw_sb[:, io, :],
                                             act_sb[:, io, ds(c0, n)],
                                             start=(io == 0),
                                             stop=(io == NIT - 1))
                        o_sb = opool.tile([128, CHUNK], bf, tag="o")
                        nc.vector.tensor_copy(o_sb[:, ds(0, n)],
                                              pd[:, ds(0, n)])
                        nc.gpsimd.dma_start(
                            out[ds(h * 128, 128), ds(s0 + c0, n)],
                            o_sb[:, ds(0, n)])
                        c0 += n

    nc.compile()
    return nc


def get_program(segs=None):
    global _last_key
    if segs is None:
        segs = _last_key
    assert segs is not None, "call kernel() first"
    if segs not in _cached:
        _cached[segs] = _build_program(segs)
    _last_key = segs
    return _cached[segs]


def _host_prep(x, gate_w, up_w, down_w, gate_wa, gate_wb, up_wa, up_wb,
               down_wa, down_wb, seg_ids):
    """Merge LoRA into weights, tile everything; returns per-core maps."""
    bf16 = ml_dtypes.bfloat16
    f32 = np.float32

    x = np.asarray(x, f32)
    # x^T tiled: [128, NKT, T]
    xt = np.ascontiguousarray(
        x.T.reshape(NKT, 128, T).transpose(1, 0, 2).astype(bf16))

    IPALL = NC_CORES * ISP                         # 11264 padded total

    def merged_gu(w, wa, wb):
        # w [I, H]; returns padded [A, H, IPALL] fp32 effective transpose
        wT = np.asarray(w, f32).T                  # [H, I]
        eff = np.zeros((A, H, IPALL), f32)
        for a in range(A):
            m = wT + np.asarray(wa[a], f32) @ np.asarray(wb[a], f32)
            for c in range(NC_CORES):
                eff[a, :, c * ISP:c * ISP + ISR] = \
                    m[:, c * ISR:(c + 1) * ISR]
        return eff

    def gu_tiles(eff):
        # per-core [A*NIT*128, NKT, 128]: rows (a, io, h_in_k)
        outs = []
        for c in range(NC_CORES):
            s = eff[:, :, c * ISP:(c + 1) * ISP]   # [A, H, ISP]
            t = s.reshape(A, NKT, 128, NIT, 128).transpose(0, 3, 2, 1, 4)
            outs.append(np.ascontiguousarray(
                t.reshape(A * NIT * 128, NKT, 128).astype(bf16)))
        return outs

    gwt_c = gu_tiles(merged_gu(gate_w, gate_wa, gate_wb))
    uwt_c = gu_tiles(merged_gu(up_w, up_wa, up_wb))

    # down: effective [A, IPALL, H]
    dT = np.asarray(down_w, f32).T                 # [I, H]
    deff = np.zeros((A, IPALL, H), f32)
    for a in range(A):
        m = dT + np.asarray(down_wa[a], f32) @ np.asarray(down_wb[a], f32)
        for c in range(NC_CORES):
            deff[a, c * ISP:c * ISP + ISR] = m[c * ISR:(c + 1) * ISR]
    dwt_c = []
    for c in range(NC_CORES):
        s = deff[:, c * ISP:(c + 1) * ISP]         # [A, ISP, H]
        t = s.reshape(A, NIT, 128, NHT, 128).transpose(0, 3, 2, 1, 4)
        dwt_c.append(np.ascontiguousarray(
            t.reshape(A * NHT * 128, NIT, 128).astype(bf16)))

    return [{"xt": xt, "gwt": gwt_c[c], "uwt": uwt_c[c], "dwt": dwt_c[c]}
            for c in range(NC_CORES)]


def kernel(x, gate_w, up_w, down_w, gate_wa, gate_wb, up_wa, up_wb,
           down_wa, down_wb, seg_ids):
    from concourse.bass_utils import run_bass_kernel_spmd

    seg_ids = np.asarray(seg_ids, np.int32)
    segs = _segments(seg_ids)
    nc = get_program(segs)
    in_maps = _host_prep(x, gate_w, up_w, down_w, gate_wa, gate_wb,
                         up_wa, up_wb, down_wa, down_wb, seg_ids)
    res = run_bass_kernel_spmd(nc, in_maps, core_ids=list(range(NC_CORES)))
    acc = np.zeros((H, T), np.float32)
    for c in range(NC_CORES):
        acc += np.asarray(res.results[c]["out"], np.float32)
    return np.ascontiguousarray(acc.T)
